# revision 1
# baseline (speedup 1.0000x reference)
"""Trainium2 Bass kernel for nn_ConvexGenerator (MoE-routed convex generator).

Expert-parallel sharding: core c owns class c's IGU weights (Wa[c], ba[c]) and
class buffer Xbuf[c]. Samples are routed by class_ids on the host (the
"all-to-all" is the host-side shard/unshard), so each core computes logits
only over its own class's columns -- the 8x headroom over the dense reference.

Per-core pipeline (all activations kept transposed, [feature, sample]):
  t  = gelu(gelu([z, onehot] @ W1 + b1) @ W2 + b2)         TensorE + ScalarE
  e  = exp(Wa_c.T @ t + ba_masked)  (bias folds mask+ba)   TensorE + ScalarE
  out= (e.T @ [X | 1]) ; num/den split off the ones column TensorE + VectorE
Softmax max-subtraction is skipped: logits are O(0.2) here, and masked columns
use a -1e9 bias so exp underflows to exactly 0.
"""

import os

import numpy as np

P = 128
B0 = 2048
LATENT = 128
C = 8
HID = 1024
D = 512
NMAX = 4096
COUNTS = np.array([1024, 1536, 2048, 2560, 3072, 3584, 3840, 4096])
NEG = -1e9
KC = HID // P     # 8 contraction chunks
NCH = NMAX // P   # 32 column chunks per class
DXT = D + 4       # X | ones | zero pad (even sizes for fp32r matmul)

_cache: dict = {}


def _build(cap: int):
    """Build + compile the per-core Tile program for sample capacity `cap`."""
    from contextlib import ExitStack

    import concourse.bacc as bacc
    import concourse.mybir as mybir
    import concourse.tile as tile

    f32 = mybir.dt.float32
    f32r = mybir.dt.float32r
    bf16 = mybir.dt.bfloat16
    f16 = mybir.dt.float16
    AF = mybir.ActivationFunctionType

    nc = bacc.Bacc("TRN2", target_bir_lowering=False, debug=False,
                   enable_asserts=False, num_devices=8)

    zT_d = nc.dram_tensor("zT", [P, cap], f16, kind="ExternalInput")
    W1z_d = nc.dram_tensor("W1z", [P, HID], f16, kind="ExternalInput")
    b1c_d = nc.dram_tensor("b1c", [P, KC], f32, kind="ExternalInput")
    W2r_d = nc.dram_tensor("W2r", [P, KC, HID], f16, kind="ExternalInput")
    b2r_d = nc.dram_tensor("b2r", [P, KC], f32, kind="ExternalInput")
    Wap_d = nc.dram_tensor("Wap", [NCH, P, HID], f16, kind="ExternalInput")
    bac_d = nc.dram_tensor("bac", [P, NCH], f32, kind="ExternalInput")
    Xp_d = nc.dram_tensor("Xp", [NCH, P, DXT], f16, kind="ExternalInput")
    out_d = nc.dram_tensor("out", [cap, D], f32, kind="ExternalOutput")

    n_st = (cap + P - 1) // P          # 128-sample tiles for the combine
    SGS = 512                          # fp32 moving-operand free-dim limit
    sgroups = [(g, min(SGS, cap - g)) for g in range(0, cap, SGS)]

    with tile.TileContext(nc) as tc, ExitStack() as ctx:
        consts = ctx.enter_context(tc.tile_pool(name="consts", bufs=1))
        wa_pool = ctx.enter_context(tc.tile_pool(name="wa", bufs=32))
        psA = ctx.enter_context(tc.tile_pool(name="psA", bufs=4, space="PSUM"))
        psL = psA
        psC = ctx.enter_context(tc.tile_pool(name="psC", bufs=2, space="PSUM"))
        outp = ctx.enter_context(tc.tile_pool(name="outp", bufs=2))

        zT_sb = consts.tile([P, cap], f16)
        nc.sync.dma_start(zT_sb[:], zT_d[:])
        W1z_sb = consts.tile([P, HID], f16)
        nc.sync.dma_start(W1z_sb[:], W1z_d[:])
        b1c_sb = consts.tile([P, KC], f32)
        nc.sync.dma_start(b1c_sb[:], b1c_d[:])
        W2_sb = consts.tile([P, KC, HID], f16)
        for k in range(KC):
            nc.sync.dma_start(W2_sb[:, k, :], W2r_d[:, k, :])
        b2r_sb = consts.tile([P, KC], f32)
        nc.sync.dma_start(b2r_sb[:], b2r_d[:])
        bac_sb = consts.tile([P, NCH], f32)
        nc.sync.dma_start(bac_sb[:], bac_d[:])
        x_all = consts.tile([P, NCH, DXT], f16)

        # ---- Phase A: cTMU (two gelu layers), activations as [hid, sample]
        h_sb = consts.tile([P, KC, cap], f16)
        t_sb = consts.tile([P, KC, cap], f16)
        for (s0, slen) in sgroups:
            for j in range(KC):
                ph = psA.tile([P, slen], f32, tag="ps_mlp")
                nc.tensor.matmul(ph[:], W1z_sb[:, j * P:(j + 1) * P],
                                 zT_sb[:, s0:s0 + slen],
                                 start=True, stop=True)
                nc.scalar.activation(h_sb[:, j, s0:s0 + slen], ph[:], AF.Gelu,
                                     bias=b1c_sb[:, j:j + 1])
            for j in range(KC):
                pt = psA.tile([P, slen], f32, tag="ps_mlp")
                for k in range(KC):
                    nc.tensor.matmul(pt[:], W2_sb[:, k, j * P:(j + 1) * P],
                                     h_sb[:, k, s0:s0 + slen],
                                     start=(k == 0), stop=(k == KC - 1))
                nc.scalar.activation(t_sb[:, j, s0:s0 + slen], pt[:], AF.Gelu,
                                     bias=b2r_sb[:, j:j + 1])

        # ---- Phase B: routed IGU logits + fused mask/bias/exp
        e_all = consts.tile([P, NCH, cap], f16)
        for i in range(NCH):
            wa_t = wa_pool.tile([P, HID], f16, tag="wa")
            nc.sync.dma_start(wa_t[:], Wap_d[i])
            nc.sync.dma_start(x_all[:, i, :], Xp_d[i])
            for (s0, slen) in sgroups:
                pl = psL.tile([P, slen], f32, tag="ps_mlp")
                for k in range(KC):
                    nc.tensor.matmul(pl[:], wa_t[:, k * P:(k + 1) * P],
                                     t_sb[:, k, s0:s0 + slen],
                                     start=(k == 0), stop=(k == KC - 1))
                nc.scalar.activation(e_all[:, i, s0:s0 + slen], pl[:], AF.Exp,
                                     bias=bac_sb[:, i:i + 1])

        # ---- Phase C: convex combination; ones-column of Xp gives the denom
        for st in range(n_st):
            sz = min(P, cap - st * P)
            pa = psC.tile([P, 256], f32, tag="pa")
            pb = psC.tile([P, 258], f32, tag="pb")
            for i in range(NCH):
                lhs = e_all[:, i, st * P:st * P + sz]
                nc.tensor.matmul(pa[:sz, :], lhs,
                                 x_all[:, i, 0:256],
                                 start=(i == 0), stop=(i == NCH - 1))
                nc.tensor.matmul(pb[:sz, :], lhs,
                                 x_all[:, i, 256:514],
                                 start=(i == 0), stop=(i == NCH - 1))
            r = outp.tile([P, 1], f32, tag="recip")
            nc.vector.reciprocal(r[:sz], pb[:sz, 256:257])
            o = outp.tile([P, D], f32, tag="out")
            nc.vector.tensor_scalar_mul(o[:sz, 0:256], pa[:sz, :], r[:sz])
            nc.vector.tensor_scalar_mul(o[:sz, 256:512], pb[:sz, 0:256], r[:sz])
            nc.sync.dma_start(out_d[st * P:st * P + sz, :], o[:sz, :])

    nc.compile()
    return nc


def _get_compiled(cap: int):
    if cap not in _cache:
        _cache[cap] = _build(cap)
    return _cache[cap]


def kernel(z, class_ids, W1, b1, W2, b2, Wa, ba, Xbuf):
    from concourse.bass_utils import run_bass_kernel_spmd

    z = np.ascontiguousarray(np.asarray(z, np.float32))
    class_ids = np.asarray(class_ids).astype(np.int64)
    W1 = np.asarray(W1, np.float32)
    b1 = np.asarray(b1, np.float32)
    W2 = np.asarray(W2, np.float32)
    b2 = np.asarray(b2, np.float32)
    Wa = np.asarray(Wa, np.float32)
    ba = np.asarray(ba, np.float32)
    Xbuf = np.asarray(Xbuf, np.float32)

    B = z.shape[0]
    order = np.argsort(class_ids, kind="stable")
    counts = np.bincount(class_ids, minlength=C)
    cap = max(64, int(-(-counts.max() // 32) * 32))

    nc = _get_compiled(cap)

    W1z = np.ascontiguousarray(W1[:LATENT]).astype(np.float16)
    W2r = np.ascontiguousarray(W2.reshape(KC, P, HID).transpose(1, 0, 2)).astype(np.float16)
    b2r = np.ascontiguousarray(b2.reshape(KC, P).T)

    in_maps = []
    idx_by_class = []
    off = 0
    for c in range(C):
        n_c = int(counts[c])
        idx = order[off:off + n_c]
        off += n_c
        idx_by_class.append(idx)

        zTc = np.zeros((P, cap), np.float16)
        zTc[:, :n_c] = z[idx].T.astype(np.float16)
        b1c = np.ascontiguousarray((b1 + W1[LATENT + c]).reshape(KC, P).T)
        Wap = np.ascontiguousarray(
            Wa[c].reshape(KC, P, NCH, P).transpose(2, 1, 0, 3).reshape(NCH, P, HID)
        ).astype(np.float16)
        bam = np.where(np.arange(NMAX) < COUNTS[c], ba[c], NEG).astype(np.float32)
        bac = np.ascontiguousarray(bam.reshape(NCH, P).T)
        Xp = np.zeros((NCH, P, DXT), np.float16)
        Xp[:, :, :D] = Xbuf[c].reshape(NCH, P, D)
        Xp[:, :, D] = 1.0

        in_maps.append({
            "zT": zTc, "W1z": W1z, "b1c": b1c, "W2r": W2r, "b2r": b2r,
            "Wap": Wap, "bac": bac, "Xp": np.ascontiguousarray(Xp),
        })

    trace = bool(os.environ.get("BASS_TRACE"))
    res = run_bass_kernel_spmd(
        nc, in_maps, core_ids=list(range(8)),
        trace=trace,
        trace_cores=list(range(8)) if trace else None,
    )
    global _last_results
    _last_results = res

    out = np.zeros((B, D), np.float32)
    for c in range(C):
        n_c = int(counts[c])
        if n_c:
            out[idx_by_class[c]] = res.results[c]["out"][:n_c]
    return out


_last_results = None



# revision 3
# speedup vs baseline: 1.5794x; 1.5794x over previous
"""Trainium2 Bass kernel for nn_ConvexGenerator (MoE-routed convex generator).

Expert-parallel with chunk-level load balancing: the 8 classes (with
128-column chunk counts [8,12,16,20,24,28,30,32], total 170) are paired
big-with-small -- (c7,c0) (c6,c1) (c5,c2) (c4,c3) -- and each pair's chunks
are split across 2 cores.  Every core runs the same program shape: slot
group 1 holds up to K1=16 chunks of the pair's big class, group 2 up to
K2=10 chunks of the small class (unused slots are dummies with bias -1e9
so exp underflows to exactly 0).  Because a class's chunks live on two
cores, each core emits *partial* softmax numerator/denominator (the
denominator rides along as a ones-column of X); the host sums the two
partials and divides.  This keeps every Wa/Xbuf byte loaded exactly once
(the dominant cost: the kernel is HBM-bound at ~360 GB/s/core).

Precision plan (rel-tol 2e-2; measured ~2e-3 end to end):
  cTMU + logits matmuls in fp8e4 with DoubleRow perf mode (2 k-tiles of
  128 per instruction, 0.5 PE cycles/row); exp output and the convex
  combination stay f16 (e/X in fp8 alone costs ~3% output error).
"""

import os

import numpy as np

P = 128
LATENT = 128
C = 8
HID = 1024
D = 512
NMAX = 4096
COUNTS = np.array([1024, 1536, 2048, 2560, 3072, 3584, 3840, 4096])
NCH = COUNTS // P          # chunks per class: [8,12,16,20,24,28,30,32]
NEG = -1e9
PAIRS = [(7, 0), (6, 1), (5, 2), (4, 3)]   # (big, small), chunk-sum 40/42/44/44
K1 = 16                    # slot-1 chunk capacity: max ceil(nch_big / 2)
K2 = 10                    # slot-2 chunk capacity: max ceil(nch_small / 2)
DXT = 516                  # X | ones | zero pad; combine free dims 256 + 258
OUTW = 514                 # num (512) | den | pad

_cache: dict = {}


def _pad32(n: int) -> int:
    return max(64, -(-n // 32) * 32)


def _build(S1: int, S2: int):
    """Per-core Tile program for group sample capacities (S1, S2)."""
    from contextlib import ExitStack

    import concourse.bacc as bacc
    import concourse.mybir as mybir
    import concourse.tile as tile

    f32 = mybir.dt.float32
    f16 = mybir.dt.float16
    f8 = mybir.dt.float8e4
    AF = mybir.ActivationFunctionType
    DR = mybir.MatmulPerfMode.DoubleRow

    nc = bacc.Bacc("TRN2", target_bir_lowering=False, debug=False,
                   enable_asserts=False, num_devices=8)

    groups = [(S1, K1), (S2, K2)]
    W18_d = nc.dram_tensor("W18", [64, 2, HID], f8, kind="ExternalInput")
    W28_d = nc.dram_tensor("W28", [P, 4, 2, HID], f8, kind="ExternalInput")
    b2c_d = nc.dram_tensor("b2c", [P, 8], f32, kind="ExternalInput")
    z8_d, b1c_d, wa_d, bac_d, x_d, out_d = [], [], [], [], [], []
    for g, (S, K) in enumerate(groups):
        z8_d.append(nc.dram_tensor(f"z8_{g}", [64, 2, S], f8, kind="ExternalInput"))
        b1c_d.append(nc.dram_tensor(f"b1c_{g}", [P, 8], f32, kind="ExternalInput"))
        wa_d.append(nc.dram_tensor(f"wa_{g}", [P, K, 4, 2, P], f8, kind="ExternalInput"))
        bac_d.append(nc.dram_tensor(f"bac_{g}", [P, K], f32, kind="ExternalInput"))
        x_d.append(nc.dram_tensor(f"x_{g}", [P, K, DXT], f16, kind="ExternalInput"))
        out_d.append(nc.dram_tensor(f"out_{g}", [S, OUTW], f16, kind="ExternalOutput"))

    def sblocks(S):
        return [(b, min(512, S - b)) for b in range(0, S, 512)]

    def sgroups(b, bl):
        return [(s, min(256, b + bl - s)) for s in range(b, b + bl, 256)]

    with tile.TileContext(nc) as tc, ExitStack() as ctx:
        consts = ctx.enter_context(tc.tile_pool(name="consts", bufs=1))
        psA = ctx.enter_context(tc.tile_pool(name="psA", bufs=3, space="PSUM"))
        psC = ctx.enter_context(tc.tile_pool(name="psC", bufs=2, space="PSUM"))
        outp = ctx.enter_context(tc.tile_pool(name="outp", bufs=3))

        # ---- input DMAs (order = stream order: small cTMU inputs first,
        # then Wa/X pieces so phase B/C start as early as possible)
        z8_sb, b1c_sb, wa_sb, bac_sb, x_sb = [], [], [], [], []
        for g, (S, K) in enumerate(groups):
            t_z = consts.tile([64, 2, S], f8, tag=f"z8_{g}")
            nc.sync.dma_start(t_z[:], z8_d[g][:])
            z8_sb.append(t_z)
            t_b1 = consts.tile([P, 8], f32, tag=f"b1c_{g}")
            nc.sync.dma_start(t_b1[:], b1c_d[g][:])
            b1c_sb.append(t_b1)
        W18_sb = consts.tile([64, 2, HID], f8)
        nc.sync.dma_start(W18_sb[:], W18_d[:])
        b2c_sb = consts.tile([P, 8], f32)
        nc.sync.dma_start(b2c_sb[:], b2c_d[:])
        W28_sb = consts.tile([P, 4, 2, HID], f8)
        nc.sync.dma_start(W28_sb[:], W28_d[:])
        for g, (S, K) in enumerate(groups):
            t_bac = consts.tile([P, K], f32, tag=f"bac_{g}")
            nc.sync.dma_start(t_bac[:], bac_d[g][:])
            bac_sb.append(t_bac)
            wa_sb.append(consts.tile([P, K, 4, 2, P], f8, tag=f"wa_{g}", name=f"wa_sb{g}"))
            x_sb.append(consts.tile([P, K, DXT], f16, tag=f"x_{g}", name=f"x_sb{g}"))
        for g, (S, K) in enumerate(groups):
            for k0 in range(0, K, 4):
                k1 = min(k0 + 4, K)
                nc.sync.dma_start(wa_sb[g][:, k0:k1], wa_d[g][:, k0:k1])
                nc.sync.dma_start(x_sb[g][:, k0:k1], x_d[g][:, k0:k1])

        # ---- Phase A: cTMU, two fp8 DoubleRow gelu layers, t in fp8
        h8, t8, e_sb = [], [], []
        for g, (S, K) in enumerate(groups):
            t_h = consts.tile([P, 4, 2, S], f8, tag=f"h8_{g}")
            h8.append(t_h)
            t_t = consts.tile([P, 4, 2, S], f8, tag=f"t8_{g}")
            t8.append(t_t)
            t_e = consts.tile([P, K, S], f16, tag=f"e_{g}")
            e_sb.append(t_e)
        for g, (S, K) in enumerate(groups):
            for (b, bl) in sblocks(S):
                for j in range(8):
                    ph = psA.tile([P, 512], f32, tag="ps")
                    for (s0, sl) in sgroups(b, bl):
                        nc.tensor.matmul(ph[:, s0 - b:s0 - b + sl],
                                         W18_sb[:, :, j * P:(j + 1) * P],
                                         z8_sb[g][:, :, s0:s0 + sl],
                                         start=True, stop=True, perf_mode=DR,
                                         skip_group_check=True)
                    nc.scalar.activation(h8[g][:, j >> 1, j & 1, b:b + bl],
                                         ph[:, :bl], AF.Gelu,
                                         bias=b1c_sb[g][:, j:j + 1])
                for j in range(8):
                    pt = psA.tile([P, 512], f32, tag="ps")
                    for (s0, sl) in sgroups(b, bl):
                        for q in range(4):
                            nc.tensor.matmul(pt[:, s0 - b:s0 - b + sl],
                                             W28_sb[:, q, :, j * P:(j + 1) * P],
                                             h8[g][:, q, :, s0:s0 + sl],
                                             start=(q == 0), stop=(q == 3),
                                             perf_mode=DR,
                                             skip_group_check=True)
                    nc.scalar.activation(t8[g][:, j >> 1, j & 1, b:b + bl],
                                         pt[:, :bl], AF.Gelu,
                                         bias=b2c_sb[:, j:j + 1])

        # ---- Phase B: routed IGU logits (fp8 DR) + fused bias/exp -> f16
        for g, (S, K) in enumerate(groups):
            for i in range(K):
                for (b, bl) in sblocks(S):
                    pl = psA.tile([P, 512], f32, tag="ps")
                    for (s0, sl) in sgroups(b, bl):
                        for q in range(4):
                            nc.tensor.matmul(pl[:, s0 - b:s0 - b + sl],
                                             wa_sb[g][:, i, q, :, :],
                                             t8[g][:, q, :, s0:s0 + sl],
                                             start=(q == 0), stop=(q == 3),
                                             perf_mode=DR,
                                             skip_group_check=True)
                    nc.scalar.activation(e_sb[g][:, i, b:b + bl], pl[:, :bl],
                                         AF.Exp, bias=bac_sb[g][:, i:i + 1])

        # ---- Phase C: partial convex combination; ones-column gives den
        for g, (S, K) in enumerate(groups):
            for st in range(-(-S // P)):
                sz = min(P, S - st * P)
                pa = psC.tile([P, 256], f32, tag="pa")
                pb = psC.tile([P, 258], f32, tag="pb")
                for i in range(K):
                    lhs = e_sb[g][:, i, st * P:st * P + sz]
                    nc.tensor.matmul(pa[:sz, :], lhs, x_sb[g][:, i, 0:256],
                                     start=(i == 0), stop=(i == K - 1))
                    nc.tensor.matmul(pb[:sz, :], lhs, x_sb[g][:, i, 256:OUTW],
                                     start=(i == 0), stop=(i == K - 1))
                o = outp.tile([P, OUTW], f16, tag="o")
                nc.vector.tensor_scalar_mul(o[:sz, 0:256], pa[:sz, :], 1.0)
                nc.vector.tensor_scalar_mul(o[:sz, 256:OUTW], pb[:sz, :], 1.0)
                nc.sync.dma_start(out_d[g][st * P:st * P + sz, :], o[:sz, :])

    nc.compile()
    return nc


def _get_compiled(key):
    if key not in _cache:
        _cache[key] = _build(*key)
    return _cache[key]


def kernel(z, class_ids, W1, b1, W2, b2, Wa, ba, Xbuf):
    import ml_dtypes
    from concourse.bass_utils import run_bass_kernel_spmd

    f8np = ml_dtypes.float8_e4m3

    def q8(a):
        return np.clip(np.asarray(a, np.float32), -240.0, 240.0).astype(f8np)

    z = np.asarray(z, np.float32)
    class_ids = np.asarray(class_ids).astype(np.int64)
    W1 = np.asarray(W1, np.float32)
    b1 = np.asarray(b1, np.float32)
    W2 = np.asarray(W2, np.float32)
    b2 = np.asarray(b2, np.float32)
    Wa = np.asarray(Wa, np.float32)
    ba = np.asarray(ba, np.float32)
    Xbuf = np.asarray(Xbuf, np.float32)

    B = z.shape[0]
    order = np.argsort(class_ids, kind="stable")
    counts = np.bincount(class_ids, minlength=C)
    idx_by_class = []
    off = 0
    for c in range(C):
        idx_by_class.append(order[off:off + int(counts[c])])
        off += int(counts[c])

    S1 = _pad32(max(int(counts[c]) for c, _ in PAIRS))
    S2 = _pad32(max(int(counts[c]) for _, c in PAIRS))
    nc = _get_compiled((S1, S2))

    # ---- shared weights, packed for DoubleRow k-tiles (2x128 per matmul)
    W18 = np.ascontiguousarray(
        q8(W1[:LATENT]).reshape(2, 64, HID).transpose(1, 0, 2))
    W28 = np.ascontiguousarray(
        q8(W2).reshape(4, 2, P, HID).transpose(2, 0, 1, 3))
    b2c = np.ascontiguousarray(b2.reshape(8, P).T)
    # Wa: [C, HID, NMAX] -> [C, p, chunk, q, kt, m]
    Wa8 = np.ascontiguousarray(
        q8(Wa).reshape(C, 4, 2, P, NMAX // P, P).transpose(0, 3, 4, 1, 2, 5))
    X16 = Xbuf.astype(np.float16).reshape(C, NMAX // P, P, D)
    ba_r = ba.reshape(C, NMAX // P, P)

    def group_arrays(c, lo, hi, S, K):
        n = int(counts[c])
        idx = idx_by_class[c]
        z8 = np.zeros((S, LATENT), f8np)
        z8[:n] = q8(z[idx])
        z8 = np.ascontiguousarray(z8.reshape(S, 2, 64).transpose(2, 1, 0))
        b1c = np.ascontiguousarray((b1 + W1[LATENT + c]).reshape(8, P).T)
        wa = np.zeros((P, K, 4, 2, P), f8np)
        wa[:, :hi - lo] = Wa8[c][:, lo:hi]
        bac = np.full((P, K), NEG, np.float32)
        bac[:, :hi - lo] = ba_r[c][lo:hi].T
        x = np.zeros((P, K, DXT), np.float16)
        x[:, :hi - lo, :D] = X16[c][lo:hi].transpose(1, 0, 2)
        x[:, :, D] = 1.0
        return {"z8": np.ascontiguousarray(z8), "b1c": b1c,
                "wa": np.ascontiguousarray(wa), "bac": bac,
                "x": np.ascontiguousarray(x)}

    in_maps = []
    for p, (cb, cs) in enumerate(PAIRS):
        hb = -(-int(NCH[cb]) // 2)
        hs = -(-int(NCH[cs]) // 2)
        for h in range(2):
            g1 = group_arrays(cb, hb * h, hb if h == 0 else int(NCH[cb]), S1, K1)
            g2 = group_arrays(cs, hs * h, hs if h == 0 else int(NCH[cs]), S2, K2)
            in_maps.append({
                "W18": W18, "W28": W28, "b2c": b2c,
                "z8_0": g1["z8"], "b1c_0": g1["b1c"], "wa_0": g1["wa"],
                "bac_0": g1["bac"], "x_0": g1["x"],
                "z8_1": g2["z8"], "b1c_1": g2["b1c"], "wa_1": g2["wa"],
                "bac_1": g2["bac"], "x_1": g2["x"],
            })

    trace = bool(os.environ.get("BASS_TRACE"))
    res = run_bass_kernel_spmd(
        nc, in_maps, core_ids=list(range(8)),
        trace=trace,
        trace_cores=list(range(8)) if trace else None,
    )
    global _last_results
    _last_results = res

    out = np.zeros((B, D), np.float32)
    for p, (cb, cs) in enumerate(PAIRS):
        for g, c in ((0, cb), (1, cs)):
            n = int(counts[c])
            if n == 0:
                continue
            acc = (res.results[2 * p][f"out_{g}"][:n].astype(np.float32)
                   + res.results[2 * p + 1][f"out_{g}"][:n].astype(np.float32))
            out[idx_by_class[c]] = acc[:, :D] / acc[:, D:D + 1]
    return out


_last_results = None
